# revision 11
# baseline (speedup 1.0000x reference)
"""BitLinear kernel for Trainium2 (8 NeuronCores, tensor-parallel).

Computes: out = x @ (sign(w) * mean(|w|, axis=1, keepdims=True)).T
  x      : [4, 2048, 4096] f32
  weight : [4096, 4096] f32
  out    : [4, 2048, 4096] f32

Strategy: shard weight rows (out features) 8-way; replicate x. Each
core computes outT[512, 8192] = (sign(w_shard) @ x.T) * scale.

Precision/speed split along the contraction (k) dim:
  - k-tiles 0..B-1 (B=16): x in bf16, signs as bf16 +-1 stationaries,
    standard matmuls (1 k-tile per 216ns MM).
  - k-tiles B..31 as F=8 pairs: x and signs in fp8 e4m3, DoubleRow
    perf-mode matmuls contracting TWO k-tiles per MM at the same
    ~215ns — 2x the bf16 rate (LDWEIGHTS hides behind the MM stream).
  The fp8 e4m3 quantization of x contributes ~2.66e-2 L2 relative
  error when applied to the whole contraction; applying it to half
  scales that by sqrt(16/32) -> ~1.87e-2, under the 2e-2 gate.

Per-row scales mean(|w|) are computed exactly on host (f64) and folded
in during PSUM eviction (scalar/vector engines), so the device never
sees the natural-layout weights at all.

Host gathers the 8 outT shards -> [4096, 8192] -> transpose -> out.
"""

import os
from contextlib import ExitStack

import numpy as np
import ml_dtypes

import concourse.bass as bass
import concourse.mybir as mybir
import concourse.tile as tile
from concourse import bacc, bass_utils

P = 128                 # SBUF partitions / PE array dim
D_IN = 4096             # contraction dim (in features)
D_OUT = 4096            # out features
M_TOT = 8192            # tokens (4*2048)
N_CORES = 8
N_SHARD = D_OUT // N_CORES      # 512 out features per core
K_TILES = D_IN // P             # 32
M_BLK = 512                     # moving free dim per matmul
M_BLKS = M_TOT // M_BLK         # 16
M_PAIRS = M_BLKS // 2           # 8 (x is loaded in 1024-token pairs)
N_TILES = N_SHARD // P          # 4

B_TILES = int(os.environ.get("BITLIN_B", "16"))   # bf16 k-tiles (from k=0)
F_PAIRS = (K_TILES - B_TILES) // 2                # fp8 DoubleRow k-pairs
assert B_TILES % 2 == 0 and B_TILES + 2 * F_PAIRS == K_TILES

_CACHE = {}
LAST_RESULTS = None  # BassKernelResults of the most recent run (for test harness)


def _install_ntff_hook():
    """Register the ctypes NTFF profiling hook under antenv.axon_hooks so
    run_bass_kernel_spmd(trace=True) can capture device profiles under axon.
    No-op if already present or the .so lacks the symbols."""
    import contextlib
    import ctypes
    import sys
    import types

    try:
        from antenv.axon_hooks import get_axon_ntff_profile_hook  # noqa: F401

        return True
    except ImportError:
        pass

    so_path = "/opt/axon/libaxon_pjrt.so"
    if not os.path.exists(so_path):
        return False
    lib = ctypes.CDLL(so_path)
    if not hasattr(lib, "axon_start_nrt_profile"):
        return False
    lib.axon_start_nrt_profile.argtypes = [
        ctypes.POINTER(ctypes.c_int64),
        ctypes.c_size_t,
    ]
    lib.axon_start_nrt_profile.restype = ctypes.c_int64
    lib.axon_stop_nrt_profile.argtypes = [ctypes.c_char_p]
    lib.axon_stop_nrt_profile.restype = ctypes.c_int64

    @contextlib.contextmanager
    def _hook(output_dir, device_ids):
        import jax

        jax.devices()
        if device_ids:
            ids = (ctypes.c_int64 * len(device_ids))(*device_ids)
            rc = lib.axon_start_nrt_profile(ids, len(device_ids))
        else:
            rc = lib.axon_start_nrt_profile(None, 0)
        if rc != 0:
            raise RuntimeError(f"axon_start_nrt_profile rc={rc}")
        try:
            yield
        finally:
            n = lib.axon_stop_nrt_profile(str(output_dir).encode())
            print(f"ntff profile: {n} file(s) written to {output_dir}")

    mod = types.ModuleType("antenv.axon_hooks")
    _state = {"hook": _hook}
    mod.set_axon_ntff_profile_hook = lambda h: _state.__setitem__("hook", h)
    mod.get_axon_ntff_profile_hook = lambda: _state["hook"]
    sys.modules["antenv.axon_hooks"] = mod
    import antenv

    antenv.axon_hooks = mod

    # artifact upload reaches for a cloud bucket that isn't available here
    bass_utils.upload_artifacts = lambda tmpdir: f"local:{tmpdir}"
    return True


def _build_nc():
    B, F = B_TILES, F_PAIRS
    PAIR_W = 2 * M_BLK  # 1024 tokens per x load pair

    nc = bacc.Bacc(
        "TRN2", target_bir_lowering=False, debug=False, num_devices=N_CORES,
        enable_partition_id=False,
    )
    # bf16 x part, pre-tiled on host: xbp[q, j, p, m] = x.T[j*128+p, q*1024+m]
    # -> every (q, j) DMA is a contiguous 256KB read, 2KB per partition.
    xbp = nc.dram_tensor(
        "xbp", [M_PAIRS, B, P, PAIR_W], mybir.dt.bfloat16, kind="ExternalInput",
    ) if B else None
    # fp8 x part in DoubleRow pair layout:
    # x8p[q, jj, p, i*1024 + m] = x.T[(B + 2*jj + i)*128 + p, q*1024+m]
    x8p = nc.dram_tensor(
        "x8p", [M_PAIRS, F, P, 2 * PAIR_W], mybir.dt.float8e4,
        kind="ExternalInput",
    ) if F else None
    # bf16 sign stationaries (+-1), transposed, adjacent k-tiles paired so
    # each chunk is 2KB/partition: sbt[jjb, p, h*512+n] = sign(w.T[(2jjb+h)*128+p, n])
    sbt = nc.dram_tensor(
        "sbt", [B // 2, P, 2 * N_SHARD], mybir.dt.bfloat16, kind="ExternalInput",
    ) if B else None
    # fp8 sign stationaries in DoubleRow slot layout, 2 k-pairs per chunk:
    # s8t[jj2, p, (jj%2)*1024 + i*512 + n] = sign(w.T[(B+2*jj+i)*128+p, n])
    s8t = nc.dram_tensor(
        "s8t", [(F + 1) // 2, P, 2 * 2 * N_SHARD], mybir.dt.float8e4,
        kind="ExternalInput",
    ) if F else None
    # exact per-out-feature scales: scl[p, ni] = mean|w[ni*128+p, :]|
    scl = nc.dram_tensor("scl", [P, N_TILES], mybir.dt.float32, kind="ExternalInput")
    outT = nc.dram_tensor(
        "outT", [N_SHARD, M_TOT], mybir.dt.float32, kind="ExternalOutput"
    )

    with tile.TileContext(nc) as tc, ExitStack() as ctx:
        spool = ctx.enter_context(tc.tile_pool(name="scales", bufs=1))
        sgpool = ctx.enter_context(tc.tile_pool(name="sign", bufs=1))
        xpool = ctx.enter_context(tc.tile_pool(name="xpair", bufs=2))
        x8pool = ctx.enter_context(tc.tile_pool(name="x8pair", bufs=2))
        opool = ctx.enter_context(tc.tile_pool(name="oblk", bufs=6))
        ppool = ctx.enter_context(tc.tile_pool(name="psum", bufs=8, space="PSUM"))

        # Two load queues, each chained in emission order so the FIFO is
        # deterministic: sync carries the bf16 x stream (+bf16 signs),
        # vector carries the fp8 stream (+scales). Stores ride scalar.
        prev_dma = {"sync": None, "gpsimd": None}

        def chain_load(eng, dst, src):
            dma = getattr(nc, eng).dma_start(dst, src)
            if prev_dma[eng] is not None:
                tile.add_dep_helper(
                    dma.ins, prev_dma[eng].ins, sync=False,
                    reason=f"{eng} DMA queue emission order",
                )
            prev_dma[eng] = dma
            return dma

        # Persistent sign tiles
        Sb = sgpool.tile([P, B * N_SHARD], mybir.dt.bfloat16, name="Sb") if B else None
        S8 = sgpool.tile([P, 2 * F, N_SHARD], mybir.dt.float8e4, name="S8") if F else None
        s_all = spool.tile([P, N_TILES], mybir.dt.float32, name="s_all")

        def issue_signs():
            """All sign/scale loads ride the gpsimd queue, ahead of the fp8
            x stream, so the sync queue is a pure bf16-x pipe and the PE's
            k-tile consumption never waits on a sign chunk mid-pair.
            Order: first two bf16 sign chunks up front (feed the earliest
            matmuls), then interleave the rest with the fp8 sign chunks."""
            seq = []
            for jjb in range(B // 2):
                seq.append(("b", jjb))
            for jj2 in range((F + 1) // 2):
                seq.insert(2 + 2 * jj2, ("f", jj2))
            for kind, idx in seq:
                if kind == "b":
                    if idx == 0:
                        # Split the first sign chunk so the very first
                        # matmul's stationary lands ~1us earlier (smaller
                        # first transfer + earlier semaphore).
                        for c4 in range(4):
                            chain_load(
                                "gpsimd",
                                Sb[:, c4 * N_SHARD // 2 : (c4 + 1) * N_SHARD // 2],
                                sbt[0, :, c4 * N_SHARD // 2 : (c4 + 1) * N_SHARD // 2],
                            )
                        continue
                    chain_load("gpsimd",
                               Sb[:, 2 * idx * N_SHARD : (2 * idx + 2) * N_SHARD],
                               sbt[idx, :, :])
                else:
                    hi = min(4 * idx + 4, 2 * F)
                    chain_load("gpsimd", S8[:, 4 * idx : hi, :],
                               s8t[idx, :, : (hi - 4 * idx) * N_SHARD])
            chain_load("gpsimd", s_all[:], scl[:, :])

        def issue_x_pair(q):
            """Load one 1024-token pair of x: bf16 chunks on sync, fp8
            chunks on gpsimd."""
            xbt = (xpool.tile([P, B * PAIR_W], mybir.dt.bfloat16, tag="xb",
                              name="xbt") if B else None)
            x8t = (x8pool.tile([P, 2 * F, PAIR_W], mybir.dt.float8e4, tag="x8",
                               name="x8t") if F else None)
            for j in range(B):
                if q == 0 and j == 0:
                    # Halve the first x chunk so matmul #0 starts sooner.
                    chain_load("sync", xbt[:, 0:M_BLK], xbp[0, 0, :, 0:M_BLK])
                    chain_load("sync", xbt[:, M_BLK:PAIR_W], xbp[0, 0, :, M_BLK:])
                    continue
                chain_load("sync", xbt[:, j * PAIR_W : (j + 1) * PAIR_W],
                           xbp[q, j, :, :])
            for jj in range(F):
                chain_load("gpsimd", x8t[:, 2 * jj : 2 * jj + 2, :],
                           x8p[q, jj, :, :])
            return xbt, x8t

        def mm_seq(pss, xbt, x8t, b, ni, t):
            """t-th matmul of the k sequence for (block b, n-tile ni):
            t in [0, B) -> bf16 k-tile t; t in [B, B+F) -> fp8 pair t-B."""
            if t < B:
                j = t
                nc.tensor.matmul(
                    pss[ni][:],
                    Sb[:, j * N_SHARD + ni * P : j * N_SHARD + (ni + 1) * P],
                    xbt[:, j * PAIR_W + b * M_BLK : j * PAIR_W + b * M_BLK + M_BLK],
                    start=(t == 0),
                    stop=(t == B + F - 1),
                )
            else:
                jj = t - B
                nc.tensor.matmul(
                    pss[ni][:],
                    S8[:, 2 * jj : 2 * jj + 2, ni * P : (ni + 1) * P],
                    x8t[:, 2 * jj : 2 * jj + 2, b * M_BLK : b * M_BLK + M_BLK],
                    start=(t == 0),
                    stop=(t == B + F - 1),
                    perf_mode=mybir.MatmulPerfMode.DoubleRow,
                )

        def evict_block(pss, mb):
            # Evictions alternate between the scalar and vector engines so
            # the per-block eviction chain (and the kernel tail) is half as
            # long. Stores ride the scalar queue; the final block's stores
            # spread over scalar/sync/gpsimd (all idle by then) so the tail
            # isn't serialized on one queue.
            last = mb == M_BLKS - 1
            store_eng = ["scalar", "sync", "gpsimd", "sync"]
            for ni in range(N_TILES):
                ot = opool.tile([P, M_BLK], mybir.dt.float32, tag="ot", name="ot")
                dst = outT[ni * P : (ni + 1) * P, mb * M_BLK : (mb + 1) * M_BLK]
                if last and ni == N_TILES - 1:
                    # The very last eviction is on the critical path: split
                    # it across both ALU engines and two idle DMA queues so
                    # the tail after the final matmul is as short as possible.
                    h = M_BLK // 2
                    nc.scalar.mul(ot[:, :h], pss[ni][:, :h], s_all[:, ni : ni + 1])
                    nc.vector.tensor_scalar_mul(
                        ot[:, h:], pss[ni][:, h:], s_all[:, ni : ni + 1]
                    )
                    nc.scalar.dma_start(
                        outT[ni * P : (ni + 1) * P,
                             mb * M_BLK : mb * M_BLK + h], ot[:, :h])
                    nc.gpsimd.dma_start(
                        outT[ni * P : (ni + 1) * P,
                             mb * M_BLK + h : (mb + 1) * M_BLK], ot[:, h:])
                    continue
                if ni % 2 == 0:
                    nc.scalar.mul(ot[:], pss[ni][:], s_all[:, ni : ni + 1])
                else:
                    nc.vector.tensor_scalar_mul(
                        ot[:], pss[ni][:], s_all[:, ni : ni + 1]
                    )
                if last:
                    getattr(nc, store_eng[ni]).dma_start(dst, ot[:])
                else:
                    nc.scalar.dma_start(dst, ot[:])

        T_SEQ = B + F  # matmuls per (block, n-tile)

        issue_signs()
        for q in range(M_PAIRS):
            xbt, x8t = issue_x_pair(q)
            if q == 0:
                # Pair 0 runs t-outer across BOTH blocks (8 PSUM banks) so
                # the PE keeps pace with the HBM-limited startup stream.
                pss2 = [
                    [
                        ppool.tile(
                            [P, M_BLK], mybir.dt.float32, tag="ps",
                            name=f"ps_{b}_{ni}",
                        )
                        for ni in range(N_TILES)
                    ]
                    for b in range(2)
                ]
                for t in range(T_SEQ):
                    for b in range(2):
                        for ni in range(N_TILES):
                            mm_seq(pss2[b], xbt, x8t, b, ni, t)
                for b in range(2):
                    evict_block(pss2[b], b)
            else:
                for b in range(2):
                    last_blk = q == M_PAIRS - 1 and b == 1
                    pss = [
                        ppool.tile(
                            [P, M_BLK], mybir.dt.float32, tag="ps", name=f"ps{ni}"
                        )
                        for ni in range(N_TILES)
                    ]
                    if last_blk:
                        # ni-outer for the final block: each n-tile's stop
                        # matmul lands early, so its eviction + store overlap
                        # the remaining matmuls instead of serializing after
                        # the last one.
                        for ni in range(N_TILES):
                            for t in range(T_SEQ):
                                mm_seq(pss, xbt, x8t, b, ni, t)
                    else:
                        for t in range(T_SEQ):
                            for ni in range(N_TILES):
                                mm_seq(pss, xbt, x8t, b, ni, t)
                    evict_block(pss, 2 * q + b)

    nc.compile()
    return nc


def kernel(x, weight):
    global LAST_RESULTS
    key = (B_TILES, F_PAIRS)
    nc = _CACHE.get(key)
    if nc is None:
        nc = _CACHE[key] = _build_nc()

    B, F = B_TILES, F_PAIRS
    x = np.asarray(x)
    weight = np.asarray(weight)
    orig_shape = x.shape

    xT = x.reshape(M_TOT, D_IN).T  # [D_IN, M_TOT] view
    # bf16 part: [M_PAIRS, B, P, 1024]
    xbp = np.ascontiguousarray(
        xT[: B * P]
        .reshape(B, P, M_PAIRS, 2 * M_BLK)
        .transpose(2, 0, 1, 3)
        .astype(ml_dtypes.bfloat16)
    ) if B else None
    # fp8 part: [M_PAIRS, F, P, 2048]; slot i = k-tile B+2jj+i
    x8p = np.ascontiguousarray(
        xT[B * P :]
        .reshape(F, 2, P, M_PAIRS, 2 * M_BLK)
        .transpose(3, 0, 2, 1, 4)
        .reshape(M_PAIRS, F, P, 4 * M_BLK)
        .astype(ml_dtypes.float8_e4m3)
    ) if F else None

    sT = np.sign(weight.T)  # [D_IN, D_OUT] f32, +-1
    s_exact = np.abs(weight.astype(np.float64)).mean(axis=1).astype(np.float32)

    in_maps = []
    for c in range(N_CORES):
        sTc = sT[:, c * N_SHARD : (c + 1) * N_SHARD]  # [D_IN, 512]
        m = {
            "scl": np.ascontiguousarray(
                s_exact[c * N_SHARD : (c + 1) * N_SHARD].reshape(N_TILES, P).T
            ),
        }
        if B:
            m["xbp"] = xbp
            m["sbt"] = np.ascontiguousarray(
                sTc[: B * P]
                .reshape(B // 2, 2, P, N_SHARD)
                .transpose(0, 2, 1, 3)
                .reshape(B // 2, P, 2 * N_SHARD)
                .astype(ml_dtypes.bfloat16)
            )
        if F:
            m["x8p"] = x8p
            s8 = (
                sTc[B * P :]
                .reshape(F, 2, P, N_SHARD)
                .transpose(0, 2, 1, 3)
                .reshape(F, P, 2 * N_SHARD)
                .astype(ml_dtypes.float8_e4m3)
            )
            if F % 2:
                pad = np.zeros((1, P, 2 * N_SHARD), ml_dtypes.float8_e4m3)
                s8 = np.concatenate([s8, pad], axis=0)
            m["s8t"] = np.ascontiguousarray(
                s8.reshape(-1, 2, P, 2 * N_SHARD)
                .transpose(0, 2, 1, 3)
                .reshape(-1, P, 4 * N_SHARD)
            )
        in_maps.append(m)

    trace = bool(int(os.environ.get("BITLIN_TRACE", "0")))
    if trace:
        trace = _install_ntff_hook()
        base = os.environ.get("BITLIN_TRACE_DIR") or None
        if base:
            import tempfile

            os.makedirs(base, exist_ok=True)
            tmpdir = tempfile.mkdtemp(dir=base)
        else:
            tmpdir = None
    else:
        tmpdir = None
    res = bass_utils.run_bass_kernel_spmd(
        nc, in_maps, core_ids=list(range(N_CORES)), trace=trace, tmpdir=tmpdir
    )
    LAST_RESULTS = res

    outT_full = np.concatenate(
        [np.asarray(res.results[c]["outT"]) for c in range(N_CORES)], axis=0
    )  # [D_OUT, M_TOT] f32
    out = np.ascontiguousarray(outT_full.T).reshape(orig_shape).astype(np.float32)
    return out


# revision 12
# speedup vs baseline: 1.1807x; 1.1807x over previous
"""BitLinear kernel for Trainium2 (8 NeuronCores, tensor-parallel).

Computes: out = x @ (sign(w) * mean(|w|, axis=1, keepdims=True)).T
  x      : [4, 2048, 4096] f32
  weight : [4096, 4096] f32
  out    : [4, 2048, 4096] f32

Strategy: shard weight rows (out features) 8-way; replicate x. Each
core computes outT[512, 8192] = (sign(w_shard) @ x.T) * scale.

Precision/speed split along the contraction (k) dim:
  - k-tiles 0..B-1 (B=16): x in bf16, signs as bf16 +-1 stationaries,
    standard matmuls (1 k-tile per 216ns MM).
  - k-tiles B..31 as F=8 pairs: x and signs in fp8 e4m3, DoubleRow
    perf-mode matmuls contracting TWO k-tiles per MM at the same
    ~215ns — 2x the bf16 rate (LDWEIGHTS hides behind the MM stream).
  The fp8 e4m3 quantization of x contributes ~2.66e-2 L2 relative
  error when applied to the whole contraction; applying it to half
  scales that by sqrt(16/32) -> ~1.87e-2, under the 2e-2 gate.

Per-row scales mean(|w|) are computed exactly on host (f64) and folded
in during PSUM eviction (scalar/vector engines), so the device never
sees the natural-layout weights at all.

Host gathers the 8 outT shards -> [4096, 8192] -> transpose -> out.
"""

import os
from contextlib import ExitStack

import numpy as np
import ml_dtypes

import concourse.bass as bass
import concourse.mybir as mybir
import concourse.tile as tile
from concourse import bacc, bass_utils

P = 128                 # SBUF partitions / PE array dim
D_IN = 4096             # contraction dim (in features)
D_OUT = 4096            # out features
M_TOT = 8192            # tokens (4*2048)
N_CORES = 8
N_SHARD = D_OUT // N_CORES      # 512 out features per core
K_TILES = D_IN // P             # 32
M_BLK = 512                     # moving free dim per matmul
M_BLKS = M_TOT // M_BLK         # 16
M_PAIRS = M_BLKS // 2           # 8 (x is loaded in 1024-token pairs)
N_TILES = N_SHARD // P          # 4

B_TILES = int(os.environ.get("BITLIN_B", "16"))   # bf16 k-tiles (from k=0)
F_PAIRS = (K_TILES - B_TILES) // 2                # fp8 DoubleRow k-pairs
assert B_TILES % 2 == 0 and B_TILES + 2 * F_PAIRS == K_TILES

_CACHE = {}
LAST_RESULTS = None  # BassKernelResults of the most recent run (for test harness)


def _install_ntff_hook():
    """Register the ctypes NTFF profiling hook under antenv.axon_hooks so
    run_bass_kernel_spmd(trace=True) can capture device profiles under axon.
    No-op if already present or the .so lacks the symbols."""
    import contextlib
    import ctypes
    import sys
    import types

    try:
        from antenv.axon_hooks import get_axon_ntff_profile_hook  # noqa: F401

        return True
    except ImportError:
        pass

    so_path = "/opt/axon/libaxon_pjrt.so"
    if not os.path.exists(so_path):
        return False
    lib = ctypes.CDLL(so_path)
    if not hasattr(lib, "axon_start_nrt_profile"):
        return False
    lib.axon_start_nrt_profile.argtypes = [
        ctypes.POINTER(ctypes.c_int64),
        ctypes.c_size_t,
    ]
    lib.axon_start_nrt_profile.restype = ctypes.c_int64
    lib.axon_stop_nrt_profile.argtypes = [ctypes.c_char_p]
    lib.axon_stop_nrt_profile.restype = ctypes.c_int64

    @contextlib.contextmanager
    def _hook(output_dir, device_ids):
        import jax

        jax.devices()
        if device_ids:
            ids = (ctypes.c_int64 * len(device_ids))(*device_ids)
            rc = lib.axon_start_nrt_profile(ids, len(device_ids))
        else:
            rc = lib.axon_start_nrt_profile(None, 0)
        if rc != 0:
            raise RuntimeError(f"axon_start_nrt_profile rc={rc}")
        try:
            yield
        finally:
            n = lib.axon_stop_nrt_profile(str(output_dir).encode())
            print(f"ntff profile: {n} file(s) written to {output_dir}")

    mod = types.ModuleType("antenv.axon_hooks")
    _state = {"hook": _hook}
    mod.set_axon_ntff_profile_hook = lambda h: _state.__setitem__("hook", h)
    mod.get_axon_ntff_profile_hook = lambda: _state["hook"]
    sys.modules["antenv.axon_hooks"] = mod
    import antenv

    antenv.axon_hooks = mod

    # artifact upload reaches for a cloud bucket that isn't available here
    bass_utils.upload_artifacts = lambda tmpdir: f"local:{tmpdir}"
    return True


def _build_nc():
    B, F = B_TILES, F_PAIRS
    PAIR_W = 2 * M_BLK  # 1024 tokens per x load pair

    nc = bacc.Bacc(
        "TRN2", target_bir_lowering=False, debug=False, num_devices=N_CORES,
        enable_partition_id=False,
    )
    # bf16 x part, pre-tiled on host: xbp[q, j, p, m] = x.T[j*128+p, q*1024+m]
    # -> every (q, j) DMA is a contiguous 256KB read, 2KB per partition.
    xbp = nc.dram_tensor(
        "xbp", [M_PAIRS, B, P, PAIR_W], mybir.dt.bfloat16, kind="ExternalInput",
    ) if B else None
    # fp8 x part in DoubleRow pair layout:
    # x8p[q, jj, p, i*1024 + m] = x.T[(B + 2*jj + i)*128 + p, q*1024+m]
    x8p = nc.dram_tensor(
        "x8p", [M_PAIRS, F, P, 2 * PAIR_W], mybir.dt.float8e4,
        kind="ExternalInput",
    ) if F else None
    # bf16 sign stationaries (+-1), transposed, adjacent k-tiles paired so
    # each chunk is 2KB/partition: sbt[jjb, p, h*512+n] = sign(w.T[(2jjb+h)*128+p, n])
    sbt = nc.dram_tensor(
        "sbt", [B // 2, P, 2 * N_SHARD], mybir.dt.bfloat16, kind="ExternalInput",
    ) if B else None
    # fp8 sign stationaries in DoubleRow slot layout, 2 k-pairs per chunk:
    # s8t[jj2, p, (jj%2)*1024 + i*512 + n] = sign(w.T[(B+2*jj+i)*128+p, n])
    s8t = nc.dram_tensor(
        "s8t", [(F + 1) // 2, P, 2 * 2 * N_SHARD], mybir.dt.float8e4,
        kind="ExternalInput",
    ) if F else None
    # exact per-out-feature scales: scl[p, ni] = mean|w[ni*128+p, :]|
    scl = nc.dram_tensor("scl", [P, N_TILES], mybir.dt.float32, kind="ExternalInput")
    outT = nc.dram_tensor(
        "outT", [N_SHARD, M_TOT], mybir.dt.float32, kind="ExternalOutput"
    )

    with tile.TileContext(nc) as tc, ExitStack() as ctx:
        spool = ctx.enter_context(tc.tile_pool(name="scales", bufs=1))
        sgpool = ctx.enter_context(tc.tile_pool(name="sign", bufs=1))
        xpool = ctx.enter_context(tc.tile_pool(name="xpair", bufs=2))
        x8pool = ctx.enter_context(tc.tile_pool(name="x8pair", bufs=2))
        opool = ctx.enter_context(tc.tile_pool(name="oblk", bufs=6))
        ppool = ctx.enter_context(tc.tile_pool(name="psum", bufs=8, space="PSUM"))

        # Two load queues, each chained in emission order so the FIFO is
        # deterministic: sync carries the bf16 x stream (+bf16 signs),
        # vector carries the fp8 stream (+scales). Stores ride scalar.
        prev_dma = {"sync": None, "gpsimd": None}

        def chain_load(eng, dst, src):
            dma = getattr(nc, eng).dma_start(dst, src)
            if prev_dma[eng] is not None:
                tile.add_dep_helper(
                    dma.ins, prev_dma[eng].ins, sync=False,
                    reason=f"{eng} DMA queue emission order",
                )
            prev_dma[eng] = dma
            return dma

        # Persistent sign tiles
        Sb = sgpool.tile([P, B * N_SHARD], mybir.dt.bfloat16, name="Sb") if B else None
        S8 = sgpool.tile([P, 2 * F, N_SHARD], mybir.dt.float8e4, name="S8") if F else None
        s_all = spool.tile([P, N_TILES], mybir.dt.float32, name="s_all")

        def issue_signs():
            """All sign/scale loads ride the gpsimd queue, ahead of the fp8
            x stream, so the sync queue is a pure bf16-x pipe and the PE's
            k-tile consumption never waits on a sign chunk mid-pair.
            Order: first two bf16 sign chunks up front (feed the earliest
            matmuls), then interleave the rest with the fp8 sign chunks."""
            seq = []
            for jjb in range(B // 2):
                seq.append(("b", jjb))
            for jj2 in range((F + 1) // 2):
                seq.insert(2 + 2 * jj2, ("f", jj2))
            for kind, idx in seq:
                if kind == "b":
                    chain_load("gpsimd",
                               Sb[:, 2 * idx * N_SHARD : (2 * idx + 2) * N_SHARD],
                               sbt[idx, :, :])
                else:
                    hi = min(4 * idx + 4, 2 * F)
                    chain_load("gpsimd", S8[:, 4 * idx : hi, :],
                               s8t[idx, :, : (hi - 4 * idx) * N_SHARD])
            chain_load("gpsimd", s_all[:], scl[:, :])

        def issue_x_pair(q):
            """Load one 1024-token pair of x: bf16 chunks on sync, fp8
            chunks on gpsimd."""
            xbt = (xpool.tile([P, B * PAIR_W], mybir.dt.bfloat16, tag="xb",
                              name="xbt") if B else None)
            x8t = (x8pool.tile([P, 2 * F, PAIR_W], mybir.dt.float8e4, tag="x8",
                               name="x8t") if F else None)
            for j in range(B):
                chain_load("sync", xbt[:, j * PAIR_W : (j + 1) * PAIR_W],
                           xbp[q, j, :, :])
            for jj in range(F):
                chain_load("gpsimd", x8t[:, 2 * jj : 2 * jj + 2, :],
                           x8p[q, jj, :, :])
            return xbt, x8t

        def mm_seq(pss, xbt, x8t, b, ni, t):
            """t-th matmul of the k sequence for (block b, n-tile ni):
            t in [0, B) -> bf16 k-tile t; t in [B, B+F) -> fp8 pair t-B."""
            if t < B:
                j = t
                nc.tensor.matmul(
                    pss[ni][:],
                    Sb[:, j * N_SHARD + ni * P : j * N_SHARD + (ni + 1) * P],
                    xbt[:, j * PAIR_W + b * M_BLK : j * PAIR_W + b * M_BLK + M_BLK],
                    start=(t == 0),
                    stop=(t == B + F - 1),
                )
            else:
                jj = t - B
                nc.tensor.matmul(
                    pss[ni][:],
                    S8[:, 2 * jj : 2 * jj + 2, ni * P : (ni + 1) * P],
                    x8t[:, 2 * jj : 2 * jj + 2, b * M_BLK : b * M_BLK + M_BLK],
                    start=(t == 0),
                    stop=(t == B + F - 1),
                    perf_mode=mybir.MatmulPerfMode.DoubleRow,
                )

        def evict_block(pss, mb):
            # Evictions alternate between the scalar and vector engines so
            # the per-block eviction chain (and the kernel tail) is half as
            # long. Stores ride the scalar queue; the final block's stores
            # spread over scalar/sync/gpsimd (all idle by then) so the tail
            # isn't serialized on one queue.
            last = mb == M_BLKS - 1
            store_eng = ["scalar", "sync", "gpsimd", "sync"]
            for ni in range(N_TILES):
                ot = opool.tile([P, M_BLK], mybir.dt.float32, tag="ot", name="ot")
                dst = outT[ni * P : (ni + 1) * P, mb * M_BLK : (mb + 1) * M_BLK]
                if ni % 2 == 0:
                    nc.scalar.mul(ot[:], pss[ni][:], s_all[:, ni : ni + 1])
                else:
                    nc.vector.tensor_scalar_mul(
                        ot[:], pss[ni][:], s_all[:, ni : ni + 1]
                    )
                if last:
                    getattr(nc, store_eng[ni]).dma_start(dst, ot[:])
                else:
                    nc.scalar.dma_start(dst, ot[:])

        T_SEQ = B + F  # matmuls per (block, n-tile)

        issue_signs()
        for q in range(M_PAIRS):
            xbt, x8t = issue_x_pair(q)
            if q == 0:
                # Pair 0 runs t-outer across BOTH blocks (8 PSUM banks) so
                # the PE keeps pace with the HBM-limited startup stream.
                pss2 = [
                    [
                        ppool.tile(
                            [P, M_BLK], mybir.dt.float32, tag="ps",
                            name=f"ps_{b}_{ni}",
                        )
                        for ni in range(N_TILES)
                    ]
                    for b in range(2)
                ]
                for t in range(T_SEQ):
                    for b in range(2):
                        for ni in range(N_TILES):
                            mm_seq(pss2[b], xbt, x8t, b, ni, t)
                for b in range(2):
                    evict_block(pss2[b], b)
            else:
                for b in range(2):
                    last_blk = q == M_PAIRS - 1 and b == 1
                    pss = [
                        ppool.tile(
                            [P, M_BLK], mybir.dt.float32, tag="ps", name=f"ps{ni}"
                        )
                        for ni in range(N_TILES)
                    ]
                    if last_blk:
                        # ni-outer for the final block: each n-tile's stop
                        # matmul lands early, so its eviction + store overlap
                        # the remaining matmuls instead of serializing after
                        # the last one.
                        for ni in range(N_TILES):
                            for t in range(T_SEQ):
                                mm_seq(pss, xbt, x8t, b, ni, t)
                    else:
                        for t in range(T_SEQ):
                            for ni in range(N_TILES):
                                mm_seq(pss, xbt, x8t, b, ni, t)
                    evict_block(pss, 2 * q + b)

    nc.compile()
    return nc


def kernel(x, weight):
    global LAST_RESULTS
    key = (B_TILES, F_PAIRS)
    nc = _CACHE.get(key)
    if nc is None:
        nc = _CACHE[key] = _build_nc()

    B, F = B_TILES, F_PAIRS
    x = np.asarray(x)
    weight = np.asarray(weight)
    orig_shape = x.shape

    xT = x.reshape(M_TOT, D_IN).T  # [D_IN, M_TOT] view
    # bf16 part: [M_PAIRS, B, P, 1024]
    xbp = np.ascontiguousarray(
        xT[: B * P]
        .reshape(B, P, M_PAIRS, 2 * M_BLK)
        .transpose(2, 0, 1, 3)
        .astype(ml_dtypes.bfloat16)
    ) if B else None
    # fp8 part: [M_PAIRS, F, P, 2048]; slot i = k-tile B+2jj+i
    x8p = np.ascontiguousarray(
        xT[B * P :]
        .reshape(F, 2, P, M_PAIRS, 2 * M_BLK)
        .transpose(3, 0, 2, 1, 4)
        .reshape(M_PAIRS, F, P, 4 * M_BLK)
        .astype(ml_dtypes.float8_e4m3)
    ) if F else None

    sT = np.sign(weight.T)  # [D_IN, D_OUT] f32, +-1
    s_exact = np.abs(weight.astype(np.float64)).mean(axis=1).astype(np.float32)

    in_maps = []
    for c in range(N_CORES):
        sTc = sT[:, c * N_SHARD : (c + 1) * N_SHARD]  # [D_IN, 512]
        m = {
            "scl": np.ascontiguousarray(
                s_exact[c * N_SHARD : (c + 1) * N_SHARD].reshape(N_TILES, P).T
            ),
        }
        if B:
            m["xbp"] = xbp
            m["sbt"] = np.ascontiguousarray(
                sTc[: B * P]
                .reshape(B // 2, 2, P, N_SHARD)
                .transpose(0, 2, 1, 3)
                .reshape(B // 2, P, 2 * N_SHARD)
                .astype(ml_dtypes.bfloat16)
            )
        if F:
            m["x8p"] = x8p
            s8 = (
                sTc[B * P :]
                .reshape(F, 2, P, N_SHARD)
                .transpose(0, 2, 1, 3)
                .reshape(F, P, 2 * N_SHARD)
                .astype(ml_dtypes.float8_e4m3)
            )
            if F % 2:
                pad = np.zeros((1, P, 2 * N_SHARD), ml_dtypes.float8_e4m3)
                s8 = np.concatenate([s8, pad], axis=0)
            m["s8t"] = np.ascontiguousarray(
                s8.reshape(-1, 2, P, 2 * N_SHARD)
                .transpose(0, 2, 1, 3)
                .reshape(-1, P, 4 * N_SHARD)
            )
        in_maps.append(m)

    trace = bool(int(os.environ.get("BITLIN_TRACE", "0")))
    if trace:
        trace = _install_ntff_hook()
        base = os.environ.get("BITLIN_TRACE_DIR") or None
        if base:
            import tempfile

            os.makedirs(base, exist_ok=True)
            tmpdir = tempfile.mkdtemp(dir=base)
        else:
            tmpdir = None
    else:
        tmpdir = None
    res = bass_utils.run_bass_kernel_spmd(
        nc, in_maps, core_ids=list(range(N_CORES)), trace=trace, tmpdir=tmpdir
    )
    LAST_RESULTS = res

    outT_full = np.concatenate(
        [np.asarray(res.results[c]["outT"]) for c in range(N_CORES)], axis=0
    )  # [D_OUT, M_TOT] f32
    out = np.ascontiguousarray(outT_full.T).reshape(orig_shape).astype(np.float32)
    return out


# revision 13
# speedup vs baseline: 1.2162x; 1.0301x over previous
"""BitLinear kernel for Trainium2 (8 NeuronCores, tensor-parallel).

Computes: out = x @ (sign(w) * mean(|w|, axis=1, keepdims=True)).T
  x      : [4, 2048, 4096] f32
  weight : [4096, 4096] f32
  out    : [4, 2048, 4096] f32

Strategy: shard weight rows (out features) 8-way; replicate x. Each
core computes outT[512, 8192] = (sign(w_shard) @ x.T) * scale.

Precision/speed split along the contraction (k) dim:
  - k-tiles 0..B-1 (B=16): x in bf16, signs as bf16 +-1 stationaries,
    standard matmuls (1 k-tile per 216ns MM).
  - k-tiles B..31 as F=8 pairs: x and signs in fp8 e4m3, DoubleRow
    perf-mode matmuls contracting TWO k-tiles per MM at the same
    ~215ns — 2x the bf16 rate (LDWEIGHTS hides behind the MM stream).
  The fp8 e4m3 quantization of x contributes ~2.66e-2 L2 relative
  error when applied to the whole contraction; applying it to half
  scales that by sqrt(16/32) -> ~1.87e-2, under the 2e-2 gate.

Per-row scales mean(|w|) are computed exactly on host (f64) and folded
in during PSUM eviction (scalar/vector engines), so the device never
sees the natural-layout weights at all.

Host gathers the 8 outT shards -> [4096, 8192] -> transpose -> out.
"""

import os
from contextlib import ExitStack

import numpy as np
import ml_dtypes

import concourse.bass as bass
import concourse.mybir as mybir
import concourse.tile as tile
from concourse import bacc, bass_utils

P = 128                 # SBUF partitions / PE array dim
D_IN = 4096             # contraction dim (in features)
D_OUT = 4096            # out features
M_TOT = 8192            # tokens (4*2048)
N_CORES = 8
N_SHARD = D_OUT // N_CORES      # 512 out features per core
K_TILES = D_IN // P             # 32
M_BLK = 512                     # moving free dim per matmul
M_BLKS = M_TOT // M_BLK         # 16
M_PAIRS = M_BLKS // 2           # 8 (x is loaded in 1024-token pairs)
N_TILES = N_SHARD // P          # 4

B_TILES = int(os.environ.get("BITLIN_B", "16"))   # bf16 k-tiles (from k=0)
F_PAIRS = (K_TILES - B_TILES) // 2                # fp8 DoubleRow k-pairs
assert B_TILES % 2 == 0 and B_TILES + 2 * F_PAIRS == K_TILES

_CACHE = {}
LAST_RESULTS = None  # BassKernelResults of the most recent run (for test harness)


def _install_ntff_hook():
    """Register the ctypes NTFF profiling hook under antenv.axon_hooks so
    run_bass_kernel_spmd(trace=True) can capture device profiles under axon.
    No-op if already present or the .so lacks the symbols."""
    import contextlib
    import ctypes
    import sys
    import types

    try:
        from antenv.axon_hooks import get_axon_ntff_profile_hook  # noqa: F401

        return True
    except ImportError:
        pass

    so_path = "/opt/axon/libaxon_pjrt.so"
    if not os.path.exists(so_path):
        return False
    lib = ctypes.CDLL(so_path)
    if not hasattr(lib, "axon_start_nrt_profile"):
        return False
    lib.axon_start_nrt_profile.argtypes = [
        ctypes.POINTER(ctypes.c_int64),
        ctypes.c_size_t,
    ]
    lib.axon_start_nrt_profile.restype = ctypes.c_int64
    lib.axon_stop_nrt_profile.argtypes = [ctypes.c_char_p]
    lib.axon_stop_nrt_profile.restype = ctypes.c_int64

    @contextlib.contextmanager
    def _hook(output_dir, device_ids):
        import jax

        jax.devices()
        if device_ids:
            ids = (ctypes.c_int64 * len(device_ids))(*device_ids)
            rc = lib.axon_start_nrt_profile(ids, len(device_ids))
        else:
            rc = lib.axon_start_nrt_profile(None, 0)
        if rc != 0:
            raise RuntimeError(f"axon_start_nrt_profile rc={rc}")
        try:
            yield
        finally:
            n = lib.axon_stop_nrt_profile(str(output_dir).encode())
            print(f"ntff profile: {n} file(s) written to {output_dir}")

    mod = types.ModuleType("antenv.axon_hooks")
    _state = {"hook": _hook}
    mod.set_axon_ntff_profile_hook = lambda h: _state.__setitem__("hook", h)
    mod.get_axon_ntff_profile_hook = lambda: _state["hook"]
    sys.modules["antenv.axon_hooks"] = mod
    import antenv

    antenv.axon_hooks = mod

    # artifact upload reaches for a cloud bucket that isn't available here
    bass_utils.upload_artifacts = lambda tmpdir: f"local:{tmpdir}"
    return True


def _build_nc():
    B, F = B_TILES, F_PAIRS
    PAIR_W = 2 * M_BLK  # 1024 tokens per x load pair

    nc = bacc.Bacc(
        "TRN2", target_bir_lowering=False, debug=False, num_devices=N_CORES,
        enable_partition_id=False,
    )
    # bf16 x part, pre-tiled on host: xbp[q, j, p, m] = x.T[j*128+p, q*1024+m]
    # -> every (q, j) DMA is a contiguous 256KB read, 2KB per partition.
    xbp = nc.dram_tensor(
        "xbp", [M_PAIRS, B, P, PAIR_W], mybir.dt.bfloat16, kind="ExternalInput",
    ) if B else None
    # fp8 x part in DoubleRow pair layout:
    # x8p[q, jj, p, i*1024 + m] = x.T[(B + 2*jj + i)*128 + p, q*1024+m]
    x8p = nc.dram_tensor(
        "x8p", [M_PAIRS, F, P, 2 * PAIR_W], mybir.dt.float8e4,
        kind="ExternalInput",
    ) if F else None
    # bf16 sign stationaries (+-1), transposed, adjacent k-tiles paired so
    # each chunk is 2KB/partition: sbt[jjb, p, h*512+n] = sign(w.T[(2jjb+h)*128+p, n])
    sbt = nc.dram_tensor(
        "sbt", [B // 2, P, 2 * N_SHARD], mybir.dt.float8e4, kind="ExternalInput",
    ) if B else None
    # fp8 sign stationaries in DoubleRow slot layout, 2 k-pairs per chunk:
    # s8t[jj2, p, (jj%2)*1024 + i*512 + n] = sign(w.T[(B+2*jj+i)*128+p, n])
    s8t = nc.dram_tensor(
        "s8t", [(F + 1) // 2, P, 2 * 2 * N_SHARD], mybir.dt.float8e4,
        kind="ExternalInput",
    ) if F else None
    # exact per-out-feature scales: scl[p, ni] = mean|w[ni*128+p, :]|
    scl = nc.dram_tensor("scl", [P, N_TILES], mybir.dt.float32, kind="ExternalInput")
    outT = nc.dram_tensor(
        "outT", [N_SHARD, M_TOT], mybir.dt.float32, kind="ExternalOutput"
    )

    with tile.TileContext(nc) as tc, ExitStack() as ctx:
        spool = ctx.enter_context(tc.tile_pool(name="scales", bufs=1))
        sgpool = ctx.enter_context(tc.tile_pool(name="sign", bufs=1))
        xpool = ctx.enter_context(tc.tile_pool(name="xpair", bufs=2))
        x8pool = ctx.enter_context(tc.tile_pool(name="x8pair", bufs=2))
        opool = ctx.enter_context(tc.tile_pool(name="oblk", bufs=6))
        ppool = ctx.enter_context(tc.tile_pool(name="psum", bufs=8, space="PSUM"))

        # Two load queues, each chained in emission order so the FIFO is
        # deterministic: sync carries the bf16 x stream (+bf16 signs),
        # vector carries the fp8 stream (+scales). Stores ride scalar.
        prev_dma = {"sync": None, "gpsimd": None}

        def chain_load(eng, dst, src):
            dma = getattr(nc, eng).dma_start(dst, src)
            if prev_dma[eng] is not None:
                tile.add_dep_helper(
                    dma.ins, prev_dma[eng].ins, sync=False,
                    reason=f"{eng} DMA queue emission order",
                )
            prev_dma[eng] = dma
            return dma

        # Persistent sign tiles
        Sb = sgpool.tile([P, B * N_SHARD], mybir.dt.float8e4, name="Sb") if B else None
        S8 = sgpool.tile([P, 2 * F, N_SHARD], mybir.dt.float8e4, name="S8") if F else None
        s_all = spool.tile([P, N_TILES], mybir.dt.float32, name="s_all")

        def issue_signs():
            """All sign/scale loads ride the gpsimd queue, ahead of the fp8
            x stream, so the sync queue is a pure bf16-x pipe and the PE's
            k-tile consumption never waits on a sign chunk mid-pair.
            Order: first two bf16 sign chunks up front (feed the earliest
            matmuls), then interleave the rest with the fp8 sign chunks."""
            seq = []
            for jjb in range(B // 2):
                seq.append(("b", jjb))
            for jj2 in range((F + 1) // 2):
                seq.insert(2 + 2 * jj2, ("f", jj2))
            for kind, idx in seq:
                if kind == "b":
                    chain_load("gpsimd",
                               Sb[:, 2 * idx * N_SHARD : (2 * idx + 2) * N_SHARD],
                               sbt[idx, :, :])
                else:
                    hi = min(4 * idx + 4, 2 * F)
                    chain_load("gpsimd", S8[:, 4 * idx : hi, :],
                               s8t[idx, :, : (hi - 4 * idx) * N_SHARD])
            chain_load("gpsimd", s_all[:], scl[:, :])

        def issue_x_pair(q):
            """Load one 1024-token pair of x: bf16 chunks on sync, fp8
            chunks on gpsimd."""
            xbt = (xpool.tile([P, B * PAIR_W], mybir.dt.bfloat16, tag="xb",
                              name="xbt") if B else None)
            x8t = (x8pool.tile([P, 2 * F, PAIR_W], mybir.dt.float8e4, tag="x8",
                               name="x8t") if F else None)
            for j in range(B):
                chain_load("sync", xbt[:, j * PAIR_W : (j + 1) * PAIR_W],
                           xbp[q, j, :, :])
            for jj in range(F):
                chain_load("gpsimd", x8t[:, 2 * jj : 2 * jj + 2, :],
                           x8p[q, jj, :, :])
            return xbt, x8t

        def mm_seq(pss, xbt, x8t, b, ni, t):
            """t-th matmul of the k sequence for (block b, n-tile ni):
            t in [0, B) -> bf16 k-tile t; t in [B, B+F) -> fp8 pair t-B."""
            if t < B:
                j = t
                nc.tensor.matmul(
                    pss[ni][:],
                    Sb[:, j * N_SHARD + ni * P : j * N_SHARD + (ni + 1) * P],
                    xbt[:, j * PAIR_W + b * M_BLK : j * PAIR_W + b * M_BLK + M_BLK],
                    start=(t == 0),
                    stop=(t == B + F - 1),
                )
            else:
                jj = t - B
                nc.tensor.matmul(
                    pss[ni][:],
                    S8[:, 2 * jj : 2 * jj + 2, ni * P : (ni + 1) * P],
                    x8t[:, 2 * jj : 2 * jj + 2, b * M_BLK : b * M_BLK + M_BLK],
                    start=(t == 0),
                    stop=(t == B + F - 1),
                    perf_mode=mybir.MatmulPerfMode.DoubleRow,
                )

        def evict_block(pss, mb):
            # Evictions alternate between the scalar and vector engines so
            # the per-block eviction chain (and the kernel tail) is half as
            # long. Stores ride the scalar queue; the final block's stores
            # spread over scalar/sync/gpsimd (all idle by then) so the tail
            # isn't serialized on one queue.
            last = mb == M_BLKS - 1
            store_eng = ["scalar", "sync", "gpsimd", "sync"]
            for ni in range(N_TILES):
                ot = opool.tile([P, M_BLK], mybir.dt.float32, tag="ot", name="ot")
                dst = outT[ni * P : (ni + 1) * P, mb * M_BLK : (mb + 1) * M_BLK]
                if ni % 2 == 0:
                    nc.scalar.mul(ot[:], pss[ni][:], s_all[:, ni : ni + 1])
                else:
                    nc.vector.tensor_scalar_mul(
                        ot[:], pss[ni][:], s_all[:, ni : ni + 1]
                    )
                if last:
                    getattr(nc, store_eng[ni]).dma_start(dst, ot[:])
                else:
                    nc.scalar.dma_start(dst, ot[:])

        T_SEQ = B + F  # matmuls per (block, n-tile)

        issue_signs()
        for q in range(M_PAIRS):
            xbt, x8t = issue_x_pair(q)
            if q == 0:
                # Pair 0 runs t-outer across BOTH blocks (8 PSUM banks) so
                # the PE keeps pace with the HBM-limited startup stream.
                pss2 = [
                    [
                        ppool.tile(
                            [P, M_BLK], mybir.dt.float32, tag="ps",
                            name=f"ps_{b}_{ni}",
                        )
                        for ni in range(N_TILES)
                    ]
                    for b in range(2)
                ]
                for t in range(T_SEQ):
                    for b in range(2):
                        for ni in range(N_TILES):
                            mm_seq(pss2[b], xbt, x8t, b, ni, t)
                for b in range(2):
                    evict_block(pss2[b], b)
            else:
                for b in range(2):
                    last_blk = q == M_PAIRS - 1 and b == 1
                    pss = [
                        ppool.tile(
                            [P, M_BLK], mybir.dt.float32, tag="ps", name=f"ps{ni}"
                        )
                        for ni in range(N_TILES)
                    ]
                    if last_blk:
                        # ni-outer for the final block: each n-tile's stop
                        # matmul lands early, so its eviction + store overlap
                        # the remaining matmuls instead of serializing after
                        # the last one.
                        for ni in range(N_TILES):
                            for t in range(T_SEQ):
                                mm_seq(pss, xbt, x8t, b, ni, t)
                    else:
                        for t in range(T_SEQ):
                            for ni in range(N_TILES):
                                mm_seq(pss, xbt, x8t, b, ni, t)
                    evict_block(pss, 2 * q + b)

    nc.compile()
    return nc


def kernel(x, weight):
    global LAST_RESULTS
    key = (B_TILES, F_PAIRS)
    nc = _CACHE.get(key)
    if nc is None:
        nc = _CACHE[key] = _build_nc()

    B, F = B_TILES, F_PAIRS
    x = np.asarray(x)
    weight = np.asarray(weight)
    orig_shape = x.shape

    xT = x.reshape(M_TOT, D_IN).T  # [D_IN, M_TOT] view
    # bf16 part: [M_PAIRS, B, P, 1024]
    xbp = np.ascontiguousarray(
        xT[: B * P]
        .reshape(B, P, M_PAIRS, 2 * M_BLK)
        .transpose(2, 0, 1, 3)
        .astype(ml_dtypes.bfloat16)
    ) if B else None
    # fp8 part: [M_PAIRS, F, P, 2048]; slot i = k-tile B+2jj+i
    x8p = np.ascontiguousarray(
        xT[B * P :]
        .reshape(F, 2, P, M_PAIRS, 2 * M_BLK)
        .transpose(3, 0, 2, 1, 4)
        .reshape(M_PAIRS, F, P, 4 * M_BLK)
        .astype(ml_dtypes.float8_e4m3)
    ) if F else None

    sT = np.sign(weight.T)  # [D_IN, D_OUT] f32, +-1
    s_exact = np.abs(weight.astype(np.float64)).mean(axis=1).astype(np.float32)

    in_maps = []
    for c in range(N_CORES):
        sTc = sT[:, c * N_SHARD : (c + 1) * N_SHARD]  # [D_IN, 512]
        m = {
            "scl": np.ascontiguousarray(
                s_exact[c * N_SHARD : (c + 1) * N_SHARD].reshape(N_TILES, P).T
            ),
        }
        if B:
            m["xbp"] = xbp
            m["sbt"] = np.ascontiguousarray(
                sTc[: B * P]
                .reshape(B // 2, 2, P, N_SHARD)
                .transpose(0, 2, 1, 3)
                .reshape(B // 2, P, 2 * N_SHARD)
                .astype(ml_dtypes.float8_e4m3)
            )
        if F:
            m["x8p"] = x8p
            s8 = (
                sTc[B * P :]
                .reshape(F, 2, P, N_SHARD)
                .transpose(0, 2, 1, 3)
                .reshape(F, P, 2 * N_SHARD)
                .astype(ml_dtypes.float8_e4m3)
            )
            if F % 2:
                pad = np.zeros((1, P, 2 * N_SHARD), ml_dtypes.float8_e4m3)
                s8 = np.concatenate([s8, pad], axis=0)
            m["s8t"] = np.ascontiguousarray(
                s8.reshape(-1, 2, P, 2 * N_SHARD)
                .transpose(0, 2, 1, 3)
                .reshape(-1, P, 4 * N_SHARD)
            )
        in_maps.append(m)

    trace = bool(int(os.environ.get("BITLIN_TRACE", "0")))
    if trace:
        trace = _install_ntff_hook()
        base = os.environ.get("BITLIN_TRACE_DIR") or None
        if base:
            import tempfile

            os.makedirs(base, exist_ok=True)
            tmpdir = tempfile.mkdtemp(dir=base)
        else:
            tmpdir = None
    else:
        tmpdir = None
    res = bass_utils.run_bass_kernel_spmd(
        nc, in_maps, core_ids=list(range(N_CORES)), trace=trace, tmpdir=tmpdir
    )
    LAST_RESULTS = res

    outT_full = np.concatenate(
        [np.asarray(res.results[c]["outT"]) for c in range(N_CORES)], axis=0
    )  # [D_OUT, M_TOT] f32
    out = np.ascontiguousarray(outT_full.T).reshape(orig_shape).astype(np.float32)
    return out
